# revision 27
# baseline (speedup 1.0000x reference)
"""Trainium2 Bass kernel for the ATripletMarginLossOHNMDM loss.

Per row i of an (B, B) input:
  sim_p      = input[i, i]
  masked     = where(target[i]==0, input[i], -big)
  sim_n[0:3] = top-3 values of masked          (hard negatives)
  d          = clip(|sim_p - sim_n|, 0.1, 0.3)
  loss       = relu(sim_n - sim_p + d)
  s          = where(loss>0, sim_n, -50)
  w          = softmax(s / 0.1)      (with max-subtraction, as jax.nn.softmax)
  out        = mean over (B, 3) of loss * w

Sharded by rows across 8 NeuronCores (1024 rows each). The rel-err
budget (2e-2) dwarfs fp8-e4m3 rounding of the candidate values
(measured 1.0e-3 end to end on the real inputs), which unlocks the
memory-bound optimum: ship the input and the 0/1 target as fp8,
16 MiB/core instead of the f32+int8 40 MiB — the HBM-per-core limit
(~358 GB/s) makes DMA the roofline at ~47us. x8 and t8 are separate
contiguous DRAM tensors (strided combined layouts measured ~9% below
line rate) DMA'd into one [128, 2, ncols] SBUF tile.

The mask runs on the otherwise-idle TensorEngine as ONE DoubleRow fp8
matmul per 512-column chunk: lhsT = [Id; -240*Id] (fp8, [128,2,128]),
rhs = [x8; t8] ([128,2,512] slices), so PSUM accumulates
masked = x - 240*t in f32 exactly (Id picks x through the array; -240
is the e4m3 max, far below any N(0,1) similarity). ~380ns/chunk,
~6.1us/tile.

The PSUM->SBUF eviction (1 elem/cycle/lane on any engine) is the
structural wall; it is split so every engine sits near the DMA rate:
  - Scalar engine: activation-Copy quarters 0-2 to bf16 (~5.9us/tile)
  - DVE: quarter 3 is evicted FUSED with its fold-1 contribution — one
    1x tensor_tensor max(PSUM q3, SBUF q1) (2048 cyc) — then fold2+
    in bf16 2x packed mode down to 512 columns and one Max8 (~5.6us)
  - GPSIMD: the other fold-1 half, max(q0, q2) in SBUF bf16 (~4.3us)
Max8 never gets a DVE fast mode, so feeding it 512 instead of 8192
columns is 16x cheaper; fold slot j aggregates columns {j, j+512, ...}
and a true top-3 element is missed only when two of them collide in one
slot (~0.5% of rows, miss substitutes the 4th-largest — sub-1e-4 effect
on the final mean).

Tile 0 is processed per 2048-column quarter (DMA -> 4 matmuls -> evict)
so PE work starts as soon as the first quarter lands. A vectorized f32
epilogue computes the margin/softmax math for all tiles at once on
[128, n_tiles, 3] (sim_p from a separately-DMA'd exact f32 diagonal),
and per-(partition, tile) partial sums are DMA'd out as [128, n_tiles].
The final mean over the 8 * 128 * n_tiles partials is computed on host.
"""

import numpy as np
import ml_dtypes

import concourse.bacc as bacc
import concourse.mybir as mybir
import concourse.tile as tile
from concourse.bass_utils import run_bass_kernel_spmd

_B = 8192          # full problem size (rows == cols)
_NCORES = 8
_P = 128           # SBUF partitions
_K = 3
_MASK_W = -240.0   # e4m3 max; masked = x - 240*t sits below any real sim
_NEG_FILL = -50.0  # reference's softmax mask fill (must match exactly)
_INV_TAU = 10.0    # 1 / 0.1
_FOLD_W = 512      # fold the row down to this width before Max8
_MM_FD = 512       # matmul chunk free dim (one DoubleRow pass)
_QCOLS = 2048      # DMA quarter for tile 0 / fold1 block size
_CHCOLS = 1024     # PSUM chunk (2 banks); 4 chunks in flight


def _build_nc(rows_per_core: int, ncols: int) -> bacc.Bacc:
    n_tiles = rows_per_core // _P
    n_q = ncols // _QCOLS
    f32 = mybir.dt.float32
    bf16 = mybir.dt.bfloat16
    fp8 = mybir.dt.float8e4
    i32 = mybir.dt.int32

    nc = bacc.Bacc()
    x8 = nc.dram_tensor("x8", [rows_per_core, ncols], fp8,
                        kind="ExternalInput")
    t8 = nc.dram_tensor("t8", [rows_per_core, ncols], fp8,
                        kind="ExternalInput")
    # wgt[:, 0, :] = Id, wgt[:, 1, :] = -240*Id  (DoubleRow stationary)
    wgt = nc.dram_tensor("wgt", [_P, 2, _P], fp8, kind="ExternalInput")
    # diag[p, t] = input diagonal element of local row t*128 + p
    diag = nc.dram_tensor("diag", [_P, n_tiles], f32, kind="ExternalInput")
    out = nc.dram_tensor("out", [_P, n_tiles], f32, kind="ExternalOutput")

    with tile.TileContext(nc) as tc:
        with (
            tc.tile_pool(name="singles", bufs=1) as singles,
            tc.tile_pool(name="io_xt", bufs=4) as io_xt,
            tc.tile_pool(name="mbuf", bufs=3) as mpool,
            tc.psum_pool(name="pp", bufs=4) as pp,
            tc.tile_pool(name="small", bufs=1) as small,
        ):
            wsb = singles.tile([_P, 2, _P], fp8)
            nc.sync.dma_start(out=wsb, in_=wgt[:, :, :])
            # top-8 per (row, tile), filled by the main loop
            vfin = singles.tile([_P, n_tiles, 8], bf16)
            diag_raw = singles.tile([_P, n_tiles], f32)

            def fold_to(m_sl, width):
                """Fold m_sl (width cols) by halves in place down to
                _FOLD_W (bf16 2x mode)."""
                w = width
                while w > _FOLD_W:
                    h = w // 2
                    nc.vector.tensor_tensor(
                        out=m_sl[:, :h], in0=m_sl[:, :h], in1=m_sl[:, h:w],
                        op=mybir.AluOpType.max)
                    w = h

            n_ch = ncols // _CHCOLS          # PSUM chunks per tile
            for t in range(n_tiles):
                rows = slice(t * _P, (t + 1) * _P)
                xt_t = io_xt.tile([_P, 2, ncols], fp8)
                m_t = mpool.tile([_P, ncols], bf16, tag="m")
                if t == 0:
                    # quarter-paced DMA so the PE starts on the first
                    # 2048 columns ~10us in instead of waiting for 2 MiB
                    for q in range(n_q):
                        qs = slice(q * _QCOLS, (q + 1) * _QCOLS)
                        nc.sync.dma_start(out=xt_t[:, 0, qs],
                                          in_=x8[rows, qs])
                        nc.sync.dma_start(out=xt_t[:, 1, qs],
                                          in_=t8[rows, qs])
                else:
                    nc.sync.dma_start(out=xt_t[:, 0, :], in_=x8[rows, :])
                    nc.sync.dma_start(out=xt_t[:, 1, :], in_=t8[rows, :])
                if t == 0:
                    # tiny; issued here so it queues behind tile 0's
                    # first quarter, not ahead of it
                    nc.sync.dma_start(out=diag_raw, in_=diag[:, :])
                # 2-bank PSUM chunks, 4 in flight, so the PE never waits
                # long on an eviction. Act evicts c0-c4 (5 ops); the DVE
                # evicts c5-c7 fused with their fold1 halves.
                pts = []
                for ch in range(n_ch):
                    pt = pp.tile([_P, _CHCOLS], f32)
                    pts.append(pt)
                    for c in range(_CHCOLS // _MM_FD):
                        col = ch * _CHCOLS + c * _MM_FD
                        nc.tensor.matmul(
                            out=pt[:, c * _MM_FD:(c + 1) * _MM_FD],
                            lhsT=wsb[:, :, :],
                            rhs=xt_t[:, :, col:col + _MM_FD],
                            start=True, stop=True,
                            perf_mode=mybir.MatmulPerfMode.DoubleRow)
                    if ch < n_ch - 3:
                        c0 = ch * _CHCOLS
                        nc.scalar.copy(out=m_t[:, c0:c0 + _CHCOLS], in_=pt)
                # fold1 pairs col j with j+4096: c5 -> m[1024:2048],
                # c6 -> m[2048:3072], c7 -> m[3072:4096]
                for k in range(3):
                    dst = _CHCOLS + k * _CHCOLS
                    nc.vector.tensor_tensor(
                        out=m_t[:, dst:dst + _CHCOLS],
                        in0=pts[n_ch - 3 + k],
                        in1=m_t[:, dst:dst + _CHCOLS],
                        op=mybir.AluOpType.max)
                # the remaining fold1 pair: m[0:1024] = max(m[0:1024],
                # m[4096:5120])
                nc.vector.tensor_tensor(
                    out=m_t[:, 0:_CHCOLS], in0=m_t[:, 0:_CHCOLS],
                    in1=m_t[:, 4 * _CHCOLS:5 * _CHCOLS],
                    op=mybir.AluOpType.max)
                # fold 4096 -> _FOLD_W by halves, then Max8
                fold_to(m_t[:, :2 * _QCOLS], 2 * _QCOLS)
                nc.vector.max(out=vfin[:, t, :], in_=m_t[:, :_FOLD_W])

            # ---- vectorized epilogue over all tiles: [128, n_tiles, 3] ----
            diag_sb = singles.tile([_P, n_tiles], f32)
            nc.vector.tensor_copy(out=diag_sb, in_=diag_raw)
            sh = [_P, n_tiles, _K]
            v = small.tile(sh, f32)                    # top-3, descending
            nc.vector.tensor_copy(out=v, in_=vfin[:, :, 0:_K])
            p_b = diag_sb.unsqueeze(-1).to_broadcast(sh)

            x = small.tile(sh, f32)                    # x = sim_n - sim_p
            nc.vector.tensor_tensor(out=x, in0=v, in1=p_b,
                                    op=mybir.AluOpType.subtract)
            # a = clip(|x|, 0.1, 0.3)  (Abs on the Scalar engine, off DVE)
            a = small.tile(sh, f32)
            nc.scalar.activation(out=a, in_=x,
                                 func=mybir.ActivationFunctionType.Abs)
            nc.vector.tensor_scalar(out=a, in0=a, scalar1=0.1, scalar2=0.3,
                                    op0=mybir.AluOpType.max,
                                    op1=mybir.AluOpType.min)
            # loss = relu(x + a); active = (x + a) > 0
            xa = small.tile(sh, f32)
            nc.vector.tensor_tensor(out=xa, in0=x, in1=a,
                                    op=mybir.AluOpType.add)
            l = small.tile(sh, f32)
            nc.vector.tensor_scalar(out=l, in0=xa, scalar1=0.0, scalar2=None,
                                    op0=mybir.AluOpType.max)
            act = small.tile(sh, i32)
            nc.vector.tensor_scalar(out=act, in0=xa, scalar1=0.0, scalar2=None,
                                    op0=mybir.AluOpType.is_gt)
            # s = where(active, v, -50)
            s = small.tile(sh, f32)
            nc.vector.memset(s, _NEG_FILL)
            nc.vector.copy_predicated(out=s, mask=act, data=v)
            # softmax(s / tau) over K, with max-subtraction (matches jax)
            smax = small.tile([_P, n_tiles], f32)
            nc.vector.reduce_max(out=smax, in_=s, axis=mybir.AxisListType.X)
            s2 = small.tile(sh, f32)
            nc.vector.tensor_tensor(out=s2, in0=s,
                                    in1=smax.unsqueeze(-1).to_broadcast(sh),
                                    op=mybir.AluOpType.subtract)
            e = small.tile(sh, f32)
            nc.scalar.activation(out=e, in_=s2,
                                 func=mybir.ActivationFunctionType.Exp,
                                 scale=_INV_TAU)
            z = small.tile([_P, n_tiles], f32)
            nc.vector.reduce_sum(out=z, in_=e, axis=mybir.AxisListType.X)
            r = small.tile([_P, n_tiles], f32)
            nc.vector.reciprocal(out=r, in_=z)
            w = small.tile(sh, f32)
            nc.vector.tensor_tensor(out=w, in0=e,
                                    in1=r.unsqueeze(-1).to_broadcast(sh),
                                    op=mybir.AluOpType.mult)
            lw = small.tile(sh, f32)
            nc.vector.tensor_tensor(out=lw, in0=l, in1=w,
                                    op=mybir.AluOpType.mult)
            out_sb = small.tile([_P, n_tiles], f32)
            nc.vector.reduce_sum(out=out_sb, in_=lw, axis=mybir.AxisListType.X)
            nc.sync.dma_start(out=out[:, :], in_=out_sb)
    nc.compile()
    return nc


def _prepare_in_maps(inp: np.ndarray, tgt: np.ndarray, ncores: int):
    b, ncols = inp.shape
    rows = b // ncores
    n_tiles = rows // _P
    fp8 = ml_dtypes.float8_e4m3
    d = np.ascontiguousarray(np.diagonal(inp)).astype(np.float32, copy=False)
    # 0/1 int32 little-endian: byte 0 of each element carries the value
    tgt_v = tgt.view(np.int8)[:, ::4]
    # DoubleRow stationary operand: [Id; -240*Id], both columns per cell
    wgt = np.zeros((_P, 2, _P), dtype=fp8)
    idx = np.arange(_P)
    wgt[idx, 0, idx] = fp8(1.0)
    wgt[idx, 1, idx] = fp8(_MASK_W)
    in_maps = []
    for c in range(ncores):
        sl = slice(c * rows, (c + 1) * rows)
        diag_c = np.ascontiguousarray(d[sl].reshape(n_tiles, _P).T)
        in_maps.append({
            "x8": inp[sl].astype(fp8),
            "t8": tgt_v[sl].astype(fp8),
            "wgt": wgt,
            "diag": diag_c,
        })
    return in_maps


_NC_CACHE = {}


def kernel(input, target):
    inp = np.asarray(input, dtype=np.float32)
    tgt = np.asarray(target, dtype=np.int32)
    b, ncols = inp.shape

    key = (b, ncols)
    nc = _NC_CACHE.get(key)
    if nc is None:
        nc = _NC_CACHE[key] = _build_nc(b // _NCORES, ncols)
    in_maps = _prepare_in_maps(inp, tgt, _NCORES)
    res = run_bass_kernel_spmd(nc, in_maps, list(range(_NCORES)))
    total = 0.0
    for r in res.results:
        total += r["out"].astype(np.float64).sum()
    return np.asarray(total / (b * _K), dtype=np.float32)


if __name__ == "__main__":
    rng = np.random.default_rng(0)
    b = _B
    x = rng.standard_normal((b, b), dtype=np.float32)
    t = rng.integers(0, 2, size=(b, b)).astype(np.int32)
    np.fill_diagonal(t, 1)
    print(kernel(x, t))


# revision 29
# speedup vs baseline: 1.1302x; 1.1302x over previous
"""Trainium2 Bass kernel for the ATripletMarginLossOHNMDM loss.

Per row i of an (B, B) input:
  sim_p      = input[i, i]
  masked     = where(target[i]==0, input[i], -big)
  sim_n[0:3] = top-3 values of masked          (hard negatives)
  d          = clip(|sim_p - sim_n|, 0.1, 0.3)
  loss       = relu(sim_n - sim_p + d)
  s          = where(loss>0, sim_n, -50)
  w          = softmax(s / 0.1)      (with max-subtraction, as jax.nn.softmax)
  out        = mean over (B, 3) of loss * w

Sharded by rows across 8 NeuronCores (1024 rows each). The rel-err
budget (2e-2) dwarfs fp8-e4m3 rounding of the candidate values
(measured 1.0e-3 end to end on the real inputs), which unlocks the
memory-bound optimum: ship the input and the 0/1 target as fp8,
16 MiB/core instead of the f32+int8 40 MiB — the HBM-per-core limit
(~358 GB/s) makes DMA the roofline at ~47us. x8 and t8 are separate
contiguous DRAM tensors (strided combined layouts measured ~9% below
line rate) DMA'd into one [128, 2, ncols] SBUF tile.

The mask runs on the otherwise-idle TensorEngine as ONE DoubleRow fp8
matmul per 512-column chunk: lhsT = [Id; -240*Id] (fp8, [128,2,128],
loaded once), rhs = [x8; t8] ([128,2,512] slices), so PSUM accumulates
masked = x - 240*t in f32 exactly (Id picks x through the array; -240
is the e4m3 max, far below any N(0,1) similarity). ~405ns/chunk,
~6.5us/tile. PSUM is managed as 2-bank [128,1024] chunks, 4 in flight,
so the PE never waits long on an eviction (4-bank quarters measured
~0.75us of PE stall per quarter).

The PSUM->SBUF eviction (1 elem/cycle/lane on any engine; GPSIMD has
no PSUM access and its tensor_tensor has no max opcode) is the
structural wall; it is split so both evicting engines sit near the DMA
rate:
  - Scalar engine: activation-Copy chunks c0-c5 to bf16 (~6.0us/tile)
  - DVE: chunks c6/c7 are evicted FUSED with their fold-1
    contribution — 1x tensor_tensor max(PSUM ck, SBUF m[ck-4096])
    (2 x 1024 cyc) — then the remaining fold-1 half and the fold chain
    in bf16 2x packed mode down to 512 columns and one Max8
    (~6.2us/tile total)
Max8 never gets a DVE fast mode, so feeding it 512 instead of 8192
columns is 16x cheaper; fold slot j aggregates columns {j, j+512, ...}
and a true top-3 element is missed only when two of them collide in one
slot (~0.5% of rows, miss substitutes the 4th-largest — sub-1e-4 effect
on the final mean).

Tile 0's DMA is issued per 2048-column quarter (x/t pairs) so the first
matmuls start ~10us in instead of behind the full 2 MiB transfer. A
vectorized f32 epilogue computes the margin/softmax math for all tiles
at once on [128, n_tiles, 3] (sim_p from a separately-DMA'd exact f32
diagonal), and per-(partition, tile) partial sums are DMA'd out as
[128, n_tiles]. The final mean over the 8 * 128 * n_tiles partials is
computed on host.

Measured on 8 axon-tunneled trn2 cores (nominal clock): 85.0us HW exec,
rel err 1.0e-3 (fp8 rounding; gate is 2e-2). History: 156.7us f32
DVE-only baseline -> 110.8us bf16 folds -> 92.6us fp8+PE -> 85.0us.
Slower-clock device pools scale all engine ops ~1.2x.
"""

import numpy as np
import ml_dtypes

import concourse.bacc as bacc
import concourse.mybir as mybir
import concourse.tile as tile
from concourse.bass_utils import run_bass_kernel_spmd

_B = 8192          # full problem size (rows == cols)
_NCORES = 8
_P = 128           # SBUF partitions
_K = 3
_MASK_W = -240.0   # e4m3 max; masked = x - 240*t sits below any real sim
_NEG_FILL = -50.0  # reference's softmax mask fill (must match exactly)
_INV_TAU = 10.0    # 1 / 0.1
_FOLD_W = 512      # fold the row down to this width before Max8
_MM_FD = 512       # matmul chunk free dim (one DoubleRow pass)
_QCOLS = 2048      # DMA quarter for tile 0 / fold1 block size
_CHCOLS = 1024     # PSUM chunk (2 banks); 4 chunks in flight


def _build_nc(rows_per_core: int, ncols: int) -> bacc.Bacc:
    n_tiles = rows_per_core // _P
    n_q = ncols // _QCOLS
    f32 = mybir.dt.float32
    bf16 = mybir.dt.bfloat16
    fp8 = mybir.dt.float8e4
    i32 = mybir.dt.int32

    nc = bacc.Bacc()
    x8 = nc.dram_tensor("x8", [rows_per_core, ncols], fp8,
                        kind="ExternalInput")
    t8 = nc.dram_tensor("t8", [rows_per_core, ncols], fp8,
                        kind="ExternalInput")
    # wgt[:, 0, :] = Id, wgt[:, 1, :] = -240*Id  (DoubleRow stationary)
    wgt = nc.dram_tensor("wgt", [_P, 2, _P], fp8, kind="ExternalInput")
    # diag[p, t] = input diagonal element of local row t*128 + p
    diag = nc.dram_tensor("diag", [_P, n_tiles], f32, kind="ExternalInput")
    out = nc.dram_tensor("out", [_P, n_tiles], f32, kind="ExternalOutput")

    with tile.TileContext(nc) as tc:
        with (
            tc.tile_pool(name="singles", bufs=1) as singles,
            tc.tile_pool(name="io_xt", bufs=4) as io_xt,
            tc.tile_pool(name="mbuf", bufs=3) as mpool,
            tc.psum_pool(name="pp", bufs=4) as pp,
            tc.tile_pool(name="small", bufs=1) as small,
        ):
            wsb = singles.tile([_P, 2, _P], fp8)
            nc.sync.dma_start(out=wsb, in_=wgt[:, :, :])
            # top-8 per (row, tile), filled by the main loop
            vfin = singles.tile([_P, n_tiles, 8], bf16)
            diag_raw = singles.tile([_P, n_tiles], f32)

            def fold_to(m_sl, width):
                """Fold m_sl (width cols) by halves in place down to
                _FOLD_W (bf16 2x mode)."""
                w = width
                while w > _FOLD_W:
                    h = w // 2
                    nc.vector.tensor_tensor(
                        out=m_sl[:, :h], in0=m_sl[:, :h], in1=m_sl[:, h:w],
                        op=mybir.AluOpType.max)
                    w = h

            n_ch = ncols // _CHCOLS          # PSUM chunks per tile
            for t in range(n_tiles):
                rows = slice(t * _P, (t + 1) * _P)
                xt_t = io_xt.tile([_P, 2, ncols], fp8)
                m_t = mpool.tile([_P, ncols], bf16, tag="m")
                if t == 0:
                    # quarter-paced DMA so the PE starts on the first
                    # 2048 columns ~10us in instead of waiting for 2 MiB
                    for q in range(n_q):
                        qs = slice(q * _QCOLS, (q + 1) * _QCOLS)
                        nc.sync.dma_start(out=xt_t[:, 0, qs],
                                          in_=x8[rows, qs])
                        nc.sync.dma_start(out=xt_t[:, 1, qs],
                                          in_=t8[rows, qs])
                else:
                    nc.sync.dma_start(out=xt_t[:, 0, :], in_=x8[rows, :])
                    nc.sync.dma_start(out=xt_t[:, 1, :], in_=t8[rows, :])
                if t == 0:
                    # tiny; issued here so it queues behind tile 0's
                    # first quarter, not ahead of it
                    nc.sync.dma_start(out=diag_raw, in_=diag[:, :])
                # 2-bank PSUM chunks, 4 in flight, so the PE never waits
                # long on an eviction. Act evicts c0-c5 (6 ops); the DVE
                # evicts c6/c7 fused with their fold1 halves.
                pts = []
                for ch in range(n_ch):
                    pt = pp.tile([_P, _CHCOLS], f32)
                    pts.append(pt)
                    for c in range(_CHCOLS // _MM_FD):
                        col = ch * _CHCOLS + c * _MM_FD
                        nc.tensor.matmul(
                            out=pt[:, c * _MM_FD:(c + 1) * _MM_FD],
                            lhsT=wsb[:, :, :],
                            rhs=xt_t[:, :, col:col + _MM_FD],
                            start=True, stop=True,
                            perf_mode=mybir.MatmulPerfMode.DoubleRow)
                    if ch < n_ch - 2:
                        c0 = ch * _CHCOLS
                        nc.scalar.copy(out=m_t[:, c0:c0 + _CHCOLS], in_=pt)
                # fold1 pairs col j with j+4096: c6 -> m[2048:3072],
                # c7 -> m[3072:4096]
                for k in range(2):
                    dst = _QCOLS + k * _CHCOLS
                    nc.vector.tensor_tensor(
                        out=m_t[:, dst:dst + _CHCOLS],
                        in0=pts[n_ch - 2 + k],
                        in1=m_t[:, dst:dst + _CHCOLS],
                        op=mybir.AluOpType.max)
                # the other fold1 half: m[0:2048] = max(m[0:2048],
                # m[4096:6144])
                nc.vector.tensor_tensor(
                    out=m_t[:, 0:_QCOLS], in0=m_t[:, 0:_QCOLS],
                    in1=m_t[:, 2 * _QCOLS:3 * _QCOLS],
                    op=mybir.AluOpType.max)
                # fold 4096 -> _FOLD_W by halves, then Max8
                fold_to(m_t[:, :2 * _QCOLS], 2 * _QCOLS)
                nc.vector.max(out=vfin[:, t, :], in_=m_t[:, :_FOLD_W])

            # ---- vectorized epilogue over all tiles: [128, n_tiles, 3] ----
            diag_sb = singles.tile([_P, n_tiles], f32)
            nc.vector.tensor_copy(out=diag_sb, in_=diag_raw)
            sh = [_P, n_tiles, _K]
            v = small.tile(sh, f32)                    # top-3, descending
            nc.vector.tensor_copy(out=v, in_=vfin[:, :, 0:_K])
            p_b = diag_sb.unsqueeze(-1).to_broadcast(sh)

            x = small.tile(sh, f32)                    # x = sim_n - sim_p
            nc.vector.tensor_tensor(out=x, in0=v, in1=p_b,
                                    op=mybir.AluOpType.subtract)
            # a = clip(|x|, 0.1, 0.3)  (Abs on the Scalar engine, off DVE)
            a = small.tile(sh, f32)
            nc.scalar.activation(out=a, in_=x,
                                 func=mybir.ActivationFunctionType.Abs)
            nc.vector.tensor_scalar(out=a, in0=a, scalar1=0.1, scalar2=0.3,
                                    op0=mybir.AluOpType.max,
                                    op1=mybir.AluOpType.min)
            # loss = relu(x + a); active = (x + a) > 0
            xa = small.tile(sh, f32)
            nc.vector.tensor_tensor(out=xa, in0=x, in1=a,
                                    op=mybir.AluOpType.add)
            l = small.tile(sh, f32)
            nc.vector.tensor_scalar(out=l, in0=xa, scalar1=0.0, scalar2=None,
                                    op0=mybir.AluOpType.max)
            act = small.tile(sh, i32)
            nc.vector.tensor_scalar(out=act, in0=xa, scalar1=0.0, scalar2=None,
                                    op0=mybir.AluOpType.is_gt)
            # s = where(active, v, -50)
            s = small.tile(sh, f32)
            nc.vector.memset(s, _NEG_FILL)
            nc.vector.copy_predicated(out=s, mask=act, data=v)
            # softmax(s / tau) over K, with max-subtraction (matches jax)
            smax = small.tile([_P, n_tiles], f32)
            nc.vector.reduce_max(out=smax, in_=s, axis=mybir.AxisListType.X)
            s2 = small.tile(sh, f32)
            nc.vector.tensor_tensor(out=s2, in0=s,
                                    in1=smax.unsqueeze(-1).to_broadcast(sh),
                                    op=mybir.AluOpType.subtract)
            e = small.tile(sh, f32)
            nc.scalar.activation(out=e, in_=s2,
                                 func=mybir.ActivationFunctionType.Exp,
                                 scale=_INV_TAU)
            z = small.tile([_P, n_tiles], f32)
            nc.vector.reduce_sum(out=z, in_=e, axis=mybir.AxisListType.X)
            r = small.tile([_P, n_tiles], f32)
            nc.vector.reciprocal(out=r, in_=z)
            w = small.tile(sh, f32)
            nc.vector.tensor_tensor(out=w, in0=e,
                                    in1=r.unsqueeze(-1).to_broadcast(sh),
                                    op=mybir.AluOpType.mult)
            lw = small.tile(sh, f32)
            nc.vector.tensor_tensor(out=lw, in0=l, in1=w,
                                    op=mybir.AluOpType.mult)
            out_sb = small.tile([_P, n_tiles], f32)
            nc.vector.reduce_sum(out=out_sb, in_=lw, axis=mybir.AxisListType.X)
            nc.sync.dma_start(out=out[:, :], in_=out_sb)
    nc.compile()
    return nc


def _prepare_in_maps(inp: np.ndarray, tgt: np.ndarray, ncores: int):
    b, ncols = inp.shape
    rows = b // ncores
    n_tiles = rows // _P
    fp8 = ml_dtypes.float8_e4m3
    d = np.ascontiguousarray(np.diagonal(inp)).astype(np.float32, copy=False)
    # 0/1 int32 little-endian: byte 0 of each element carries the value
    tgt_v = tgt.view(np.int8)[:, ::4]
    # DoubleRow stationary operand: [Id; -240*Id], both columns per cell
    wgt = np.zeros((_P, 2, _P), dtype=fp8)
    idx = np.arange(_P)
    wgt[idx, 0, idx] = fp8(1.0)
    wgt[idx, 1, idx] = fp8(_MASK_W)
    in_maps = []
    for c in range(ncores):
        sl = slice(c * rows, (c + 1) * rows)
        diag_c = np.ascontiguousarray(d[sl].reshape(n_tiles, _P).T)
        in_maps.append({
            "x8": inp[sl].astype(fp8),
            "t8": tgt_v[sl].astype(fp8),
            "wgt": wgt,
            "diag": diag_c,
        })
    return in_maps


_NC_CACHE = {}


def kernel(input, target):
    inp = np.asarray(input, dtype=np.float32)
    tgt = np.asarray(target, dtype=np.int32)
    b, ncols = inp.shape

    key = (b, ncols)
    nc = _NC_CACHE.get(key)
    if nc is None:
        nc = _NC_CACHE[key] = _build_nc(b // _NCORES, ncols)
    in_maps = _prepare_in_maps(inp, tgt, _NCORES)
    res = run_bass_kernel_spmd(nc, in_maps, list(range(_NCORES)))
    total = 0.0
    for r in res.results:
        total += r["out"].astype(np.float64).sum()
    return np.asarray(total / (b * _K), dtype=np.float32)


if __name__ == "__main__":
    rng = np.random.default_rng(0)
    b = _B
    x = rng.standard_normal((b, b), dtype=np.float32)
    t = rng.integers(0, 2, size=(b, b)).astype(np.int32)
    np.fill_diagonal(t, 1)
    print(kernel(x, t))
